# revision 11
# baseline (speedup 1.0000x reference)
"""Trainium2 Bass kernel for ClipPeakMatcher (NMS-style frame matching).

Problem (hardcoded shapes): B=4 clips, N=20 instances, T=8 frames,
P=25600 ref points, C=40 classes.  reference() sorts instances by mean
area, computes normalized center distances dist[n,p], then per frame
sequentially claims points: pos = (inner if any else argmin) & active,
writing label / one-hot score / id per claimed point and suppressing
claimed points for later instances.

Device strategy (8 cores, full I/O):
- 32 independent (b,t) frames -> 4 frames per core.
- Layout: partition = frame(4) x chunk(32), free = 800 points
  (p = chunk*800 + w).  All elementwise work is [128, 800].
- Per instance n: dist_n = square(rx*iwx - cx*iwx) + square(...) via
  ScalarE activations + GPSIMD add; u_n = relu(1 - 2*dist_n) (score).
- Sequential claim loop (the only serial part): d_eff = dist_n + claimed;
  per-frame global min via free-reduce + 32x32 stream-transpose trick;
  pos = d_eff <= max(min, 0.5) (act-gated), claimed += pos*1e9,
  accumulate lab+1 / id+1 / score.
  (pos = d_eff <= max(min,0.5) is exactly inner-or-argmin: if min<0.5 it
  selects d<0.5 [<= vs < immaterial, no point sits exactly at 0.5]; else
  it selects only the argmin point.)
- md[p,c] = (labA == c+1) * scoreA built densely, DMA'd out in quarters.
- ml/mi returned as f32 accumulators; host does (acc-1).astype(int32).

Numerics: float64 margin analysis showed the discrete decisions
(sort order, 0.5 threshold, argmin) are stable under reciprocal-multiply
vs divide and fused scale/bias rounding; outputs match the jax reference
bitwise for ml/mi and to ~4e-6 on md scores.
"""
import sys

import numpy as np

sys.path.insert(0, "/opt/trn_rl_repo")

import concourse.bass as bass  # noqa: E402
import concourse.mybir as mybir  # noqa: E402
from concourse import bacc  # noqa: E402
from concourse.mybir import ActivationFunctionType as afunc  # noqa: E402
from concourse.mybir import AluOpType as alu  # noqa: E402
from concourse.tile import TileContext  # noqa: E402
from concourse.bass_utils import run_bass_kernel_spmd  # noqa: E402

F32 = mybir.dt.float32
U8 = mybir.dt.uint8

B, N, T = 4, 20, 8
P, C = 25600, 40
NCORES = 8
G = 4          # frames per core
CH = 32        # chunks per frame (partition groups of 32)
W = 800        # free dim: points per chunk
NQ = 4         # md DMA quarters
WQ = W // NQ   # 200 points per quarter
INNER_TH = 0.5
MIN_WH = 0.05
SUPPRESS = 1e9

# param rows in the packed [128, 8*N] tensor
P_SXS, P_SXB, P_SYS, P_SYB, P_ACT, P_ACTM1, P_LAB, P_ID = range(8)

_CACHE = {}


def _build_program():
    nc = bacc.Bacc("TRN2", target_bir_lowering=False, debug=True)
    rxy_d = nc.dram_tensor("rxy", [128, 2 * W], F32, kind="ExternalInput")
    par_d = nc.dram_tensor("par", [128, 8 * N], F32, kind="ExternalInput")
    oml_d = nc.dram_tensor("oml", [G, P], F32, kind="ExternalOutput")
    omd_d = nc.dram_tensor("omd", [G, P, C], F32, kind="ExternalOutput")

    with TileContext(nc) as tc:
        with (
            tc.tile_pool(name="const", bufs=1) as cpool,
            tc.tile_pool(name="state", bufs=1) as spool,
            tc.tile_pool(name="dist", bufs=5) as dpool,
            tc.tile_pool(name="work", bufs=3) as wpool,
            tc.tile_pool(name="small", bufs=2) as mpool,
            tc.tile_pool(name="md", bufs=3) as mdpool,
        ):
            rxy = cpool.tile([128, 2 * W], F32)
            par = cpool.tile([128, 8 * N], F32)
            nc.sync.dma_start(par[:], par_d[:])
            nc.sync.dma_start(rxy[:, 0:W], rxy_d[:, 0:W])
            nc.scalar.dma_start(rxy[:, W:2 * W], rxy_d[:, W:2 * W])
            rx = rxy[:, 0:W]
            ry = rxy[:, W:2 * W]

            def pcol(k, n):
                return par[:, k * N + n:k * N + n + 1]

            scoreA = spool.tile([128, W], F32)
            codeA = spool.tile([128, W], F32)
            nc.gpsimd.memset(scoreA[:], 0.0)
            nc.vector.memset(codeA[:], 0.0)

            # phase 1 producers + serial claim loop (Tile overlaps them)
            dists, us = [], []
            for n in range(N):
                dx2 = wpool.tile([128, W], F32, tag="dx2")
                nc.scalar.activation(dx2[:], rx, afunc.Square,
                                     bias=pcol(P_SXB, n), scale=pcol(P_SXS, n))
                dn = dpool.tile([128, W], F32, tag="dist")
                nc.scalar.activation(dn[:], ry, afunc.Square,
                                     bias=pcol(P_SYB, n), scale=pcol(P_SYS, n))
                nc.gpsimd.tensor_tensor(dn[:], dn[:], dx2[:], alu.add)
                un = dpool.tile([128, W], F32, tag="u")
                nc.scalar.activation(un[:], dn[:], afunc.Relu,
                                     bias=1.0, scale=-2.0)
                dists.append(dn)
                us.append(un)

            for n in range(N):
                dn, un = dists[n], us[n]
                deff = wpool.tile([128, W], F32, tag="deff")
                nc.vector.scalar_tensor_tensor(
                    out=deff[:], in0=codeA[:], scalar=SUPPRESS, in1=dn[:],
                    op0=alu.mult, op1=alu.add)
                mpart = mpool.tile([128, 1], F32, tag="mpart")
                nc.vector.tensor_reduce(out=mpart[:], in_=deff[:],
                                        axis=mybir.AxisListType.X, op=alu.min)
                # per-32-partition-group (= per-frame) min, broadcast back:
                # replicate col -> 32, block-transpose, free-min
                sbb = mpool.tile([128, 32], F32, tag="sbb")
                nc.vector.tensor_copy(sbb[:], mpart[:, 0:1].broadcast_to([128, 32]))
                t1 = mpool.tile([128, 32], F32, tag="t1")
                nc.vector.transpose(t1[:], sbb[:])
                mb = mpool.tile([128, 1], F32, tag="mb")
                nc.vector.tensor_reduce(out=mb[:], in_=t1[:],
                                        axis=mybir.AxisListType.X, op=alu.min)
                # threshold = max(min, 0.5) * (2*act-1): act=0 -> negative,
                # no point selected (deff >= 0 > th)
                th = mpool.tile([128, 1], F32, tag="th")
                nc.vector.scalar_tensor_tensor(
                    out=th[:], in0=mb[:], scalar=INNER_TH, in1=pcol(P_ACT, n),
                    op0=alu.max, op1=alu.mult)
                pos = wpool.tile([128, W], U8, tag="pos")
                nc.gpsimd.tensor_scalar(pos[:], deff[:], th[:, 0:1], None,
                                        op0=alu.is_le)
                nc.vector.scalar_tensor_tensor(
                    out=codeA[:], in0=pos[:], scalar=pcol(P_LAB, n), in1=codeA[:],
                    op0=alu.mult, op1=alu.add)
                sc = wpool.tile([128, W], F32, tag="sc")
                nc.gpsimd.tensor_tensor(sc[:], pos[:], un[:], alu.mult)
                nc.gpsimd.tensor_tensor(scoreA[:], scoreA[:], sc[:], alu.add)

            nc.sync.dma_start(
                oml_d.rearrange("g (ch w) -> (g ch) w", ch=CH), codeA[:])
            labU8 = spool.tile([128, W], U8)
            nc.vector.tensor_copy(labU8[:], codeA[:])

            # md[p, c] = (labA == c+1) * scoreA, built per quarter
            omd_v = omd_d.rearrange("g (ch q wq) c -> (g ch) q (wq c)",
                                    ch=CH, q=NQ, wq=WQ)
            for q in range(NQ):
                mdt = mdpool.tile([128, WQ * C], F32, tag="mdt")
                mdt3 = mdt[:].rearrange("p (wq c) -> p wq c", c=C)
                lab_q = labU8[:, q * WQ:(q + 1) * WQ]
                sco_q = scoreA[:, q * WQ:(q + 1) * WQ]
                for c in range(C):
                    if c % 5 < 3:
                        nc.vector.scalar_tensor_tensor(
                            out=mdt3[:, :, c], in0=lab_q, scalar=float(c + 1),
                            in1=sco_q, op0=alu.is_equal, op1=alu.mult)
                    else:
                        eqt = wpool.tile([128, WQ], F32, tag="eqt")
                        nc.gpsimd.tensor_scalar(eqt[:], lab_q, float(c + 1),
                                                None, op0=alu.is_equal)
                        nc.gpsimd.tensor_tensor(mdt3[:, :, c], eqt[:], sco_q,
                                                alu.mult)
                nseg = 4 if q == NQ - 1 else 2
                seg = WQ * C // nseg
                for s in range(nseg):
                    deng = nc.sync if s % 2 == 0 else nc.scalar
                    deng.dma_start(omd_v[:, q, s * seg:(s + 1) * seg],
                                   mdt[:, s * seg:(s + 1) * seg])

    nc.compile()
    return nc


def _preprocess(gt_labels, gt_boxes, gt_ids):
    """Mirror of reference preprocessing in f32 numpy (margin-validated)."""
    f = np.float32
    b = np.asarray(gt_boxes, dtype=f)
    labels0 = np.asarray(gt_labels).astype(np.int32)
    ids0 = np.asarray(gt_ids).astype(np.int32)
    x1, y1, x2, y2 = b[..., 0], b[..., 1], b[..., 2], b[..., 3]
    cx = (x1 + x2) * f(0.5)
    cy = (y1 + y2) * f(0.5)
    w = x2 - x1
    h = y2 - y1
    area = (w * h).mean(-1)
    order = np.argsort(area, axis=1, kind="stable")
    take = np.take_along_axis
    cx = take(cx, order[:, :, None], 1)
    cy = take(cy, order[:, :, None], 1)
    w = take(w, order[:, :, None], 1)
    h = take(h, order[:, :, None], 1)
    labels = take(labels0, order, 1)
    ids = take(ids0, order[:, :, None], 1)
    valid = ((w > 0) & (h > 0)).any(-1) & (labels >= 0)          # [B,N]
    act = valid[:, :, None] & (ids != -1)                         # [B,N,T]
    iw = f(1.0) / np.maximum(w, f(MIN_WH))
    ih = f(1.0) / np.maximum(h, f(MIN_WH))
    return cx, cy, iw, ih, labels, ids, act


def build_in_maps(gt_labels, gt_boxes, gt_ids, ref_points):
    cx, cy, iw, ih, labels, ids, act = _preprocess(gt_labels, gt_boxes, gt_ids)
    rp = np.asarray(ref_points, dtype=np.float32)
    rx = np.tile(rp[:, 0].reshape(CH, W), (G, 1))
    ry = np.tile(rp[:, 1].reshape(CH, W), (G, 1))
    rxy = np.ascontiguousarray(np.concatenate([rx, ry], axis=1))

    in_maps = []
    for core in range(NCORES):
        par = np.zeros((8, 128, N), np.float32)
        for g in range(G):
            fi = core * G + g
            bb, tt = fi // T, fi % T
            rows = slice(g * CH, (g + 1) * CH)
            par[P_SXS, rows, :] = iw[bb, :, tt]
            par[P_SXB, rows, :] = -cx[bb, :, tt] * iw[bb, :, tt]
            par[P_SYS, rows, :] = ih[bb, :, tt]
            par[P_SYB, rows, :] = -cy[bb, :, tt] * ih[bb, :, tt]
            a = act[bb, :, tt].astype(np.float32)
            par[P_ACT, rows, :] = 2.0 * a - 1.0
            par[P_LAB, rows, :] = (
                labels[bb].astype(np.float32) + 1.0
                + (ids[bb, :, tt].astype(np.float32) + 1.0) * np.float32(1 / 256)
            )
        in_maps.append(dict(rxy=rxy,
                            par=np.ascontiguousarray(
                                par.transpose(1, 0, 2).reshape(128, 8 * N))))
    return in_maps


def kernel(gt_labels, gt_boxes, gt_ids, ref_points, spatial_shapes):
    if "nc" not in _CACHE:
        _CACHE["nc"] = _build_program()
    nc = _CACHE["nc"]
    in_maps = build_in_maps(gt_labels, gt_boxes, gt_ids, ref_points)
    res = run_bass_kernel_spmd(nc, in_maps, core_ids=list(range(NCORES)),
                               **_CACHE.get("run_kwargs", {}))
    _CACHE["last_result"] = res

    ml = np.empty((B, T, P), np.int32)
    mi = np.empty((B, T, P), np.int32)
    md = np.empty((B, T, P, C), np.float32)
    for core in range(NCORES):
        out = res.results[core]
        for g in range(G):
            fi = core * G + g
            bb, tt = fi // T, fi % T
            code = out["oml"][g].astype(np.float64)
            lab = np.floor(code)
            ml[bb, tt] = lab.astype(np.int32) - 1
            mi[bb, tt] = np.rint((code - lab) * 256).astype(np.int32) - 1
            md[bb, tt] = out["omd"][g]
    return ml, md, mi


# revision 13
# speedup vs baseline: 1.0554x; 1.0554x over previous
"""Trainium2 Bass kernel for ClipPeakMatcher (NMS-style frame matching).

Problem (hardcoded shapes): B=4 clips, N=20 instances, T=8 frames,
P=25600 ref points, C=40 classes.  reference() sorts instances by mean
area, computes normalized center distances dist[n,p], then per frame
sequentially claims points: pos = (inner if any else argmin) & active,
writing label / one-hot score / id per claimed point and suppressing
claimed points for later instances.

Device strategy (8 cores, full I/O):
- 32 independent (b,t) frames -> 4 frames per core.
- Layout: partition = frame(4) x chunk(32), free = 800 points
  (p = chunk*800 + w).  All elementwise work is [128, 800].
- Per instance n: dist_n = square(rx*iwx - cx*iwx) + square(...) via
  ScalarE activations + GPSIMD add; u_n = relu(1 - 2*dist_n) (score).
- Sequential claim loop (the only serial part): d_eff = dist_n + claimed;
  per-frame global min via free-reduce + 32x32 stream-transpose trick;
  pos = d_eff <= max(min, 0.5) (act-gated), claimed += pos*1e9,
  accumulate lab+1 / id+1 / score.
  (pos = d_eff <= max(min,0.5) is exactly inner-or-argmin: if min<0.5 it
  selects d<0.5 [<= vs < immaterial, no point sits exactly at 0.5]; else
  it selects only the argmin point.)
- md[p,c] = (labA == c+1) * scoreA built densely, DMA'd out in quarters.
- ml/mi returned as f32 accumulators; host does (acc-1).astype(int32).

Numerics: float64 margin analysis showed the discrete decisions
(sort order, 0.5 threshold, argmin) are stable under reciprocal-multiply
vs divide and fused scale/bias rounding; outputs match the jax reference
bitwise for ml/mi and to ~4e-6 on md scores.
"""
import sys

import numpy as np

sys.path.insert(0, "/opt/trn_rl_repo")

import concourse.bass as bass  # noqa: E402
import concourse.mybir as mybir  # noqa: E402
from concourse import bacc  # noqa: E402
from concourse.mybir import ActivationFunctionType as afunc  # noqa: E402
from concourse.mybir import AluOpType as alu  # noqa: E402
from concourse.tile import TileContext  # noqa: E402
from concourse.bass_utils import run_bass_kernel_spmd  # noqa: E402

F32 = mybir.dt.float32
U8 = mybir.dt.uint8

B, N, T = 4, 20, 8
P, C = 25600, 40
NCORES = 8
G = 4          # frames per core
CH = 32        # chunks per frame (partition groups of 32)
W = 800        # free dim: points per chunk
NQ = 4         # md DMA quarters
WQ = W // NQ   # 200 points per quarter
INNER_TH = 0.5
MIN_WH = 0.05
SUPPRESS = 1e9

# param rows in the packed [128, 8*N] tensor
P_SXS, P_SXB, P_SYS, P_SYB, P_ACT, P_ACTM1, P_LAB, P_ID = range(8)

_CACHE = {}


def _build_program():
    nc = bacc.Bacc("TRN2", target_bir_lowering=False, debug=True)
    rxy_d = nc.dram_tensor("rxy", [128, 2 * W], F32, kind="ExternalInput")
    par_d = nc.dram_tensor("par", [128, 8 * N], F32, kind="ExternalInput")
    oml_d = nc.dram_tensor("oml", [G, P], F32, kind="ExternalOutput")
    omd_d = nc.dram_tensor("omd", [G, P, C], F32, kind="ExternalOutput")

    with TileContext(nc) as tc:
        with (
            tc.tile_pool(name="const", bufs=1) as cpool,
            tc.tile_pool(name="state", bufs=1) as spool,
            tc.tile_pool(name="dist", bufs=5) as dpool,
            tc.tile_pool(name="work", bufs=3) as wpool,
            tc.tile_pool(name="small", bufs=2) as mpool,
            tc.tile_pool(name="md", bufs=3) as mdpool,
        ):
            rxy = cpool.tile([128, 2 * W], F32)
            par = cpool.tile([128, 8 * N], F32)
            nc.sync.dma_start(par[:], par_d[:])
            nc.sync.dma_start(rxy[:, 0:W], rxy_d[:, 0:W])
            nc.scalar.dma_start(rxy[:, W:2 * W], rxy_d[:, W:2 * W])
            rx = rxy[:, 0:W]
            ry = rxy[:, W:2 * W]

            def pcol(k, n):
                return par[:, k * N + n:k * N + n + 1]

            scoreA = spool.tile([128, W], F32)
            codeA = spool.tile([128, W], F32)
            nc.gpsimd.memset(scoreA[:], 0.0)
            nc.vector.memset(codeA[:], 0.0)

            # interleave dist producers with the serial claim loop so the
            # scheduler prioritizes the chain (producers run ~2 steps ahead)
            dists = []

            def emit_dist(n):
                dx2 = wpool.tile([128, W], F32, tag="dx2")
                nc.scalar.activation(dx2[:], rx, afunc.Square,
                                     bias=pcol(P_SXB, n), scale=pcol(P_SXS, n))
                dn = dpool.tile([128, W], F32, tag="dist")
                nc.scalar.activation(dn[:], ry, afunc.Square,
                                     bias=pcol(P_SYB, n), scale=pcol(P_SYS, n))
                nc.gpsimd.tensor_tensor(dn[:], dn[:], dx2[:], alu.add)
                dists.append(dn)

            emit_dist(0)
            emit_dist(1)
            for n in range(N):
                if n + 2 < N:
                    emit_dist(n + 2)
                dn = dists[n]
                deff = wpool.tile([128, W], F32, tag="deff")
                nc.vector.scalar_tensor_tensor(
                    out=deff[:], in0=codeA[:], scalar=SUPPRESS, in1=dn[:],
                    op0=alu.mult, op1=alu.add)
                mpart = mpool.tile([128, 1], F32, tag="mpart")
                nc.vector.tensor_reduce(out=mpart[:], in_=deff[:],
                                        axis=mybir.AxisListType.X, op=alu.min)
                # per-32-partition-group (= per-frame) min, broadcast back:
                # replicate col -> 32, block-transpose, free-min
                sbb = mpool.tile([128, 32], F32, tag="sbb")
                nc.vector.tensor_copy(sbb[:], mpart[:, 0:1].broadcast_to([128, 32]))
                t1 = mpool.tile([128, 32], F32, tag="t1")
                nc.vector.transpose(t1[:], sbb[:])
                mb = mpool.tile([128, 1], F32, tag="mb")
                nc.vector.tensor_reduce(out=mb[:], in_=t1[:],
                                        axis=mybir.AxisListType.X, op=alu.min)
                # threshold = max(min, 0.5) * (2*act-1): act=0 -> negative,
                # no point selected (deff >= 0 > th)
                th = mpool.tile([128, 1], F32, tag="th")
                nc.vector.scalar_tensor_tensor(
                    out=th[:], in0=mb[:], scalar=INNER_TH, in1=pcol(P_ACT, n),
                    op0=alu.max, op1=alu.mult)
                pos = wpool.tile([128, W], U8, tag="pos")
                nc.gpsimd.tensor_scalar(pos[:], deff[:], th[:, 0:1], None,
                                        op0=alu.is_le)
                nc.vector.scalar_tensor_tensor(
                    out=codeA[:], in0=pos[:], scalar=pcol(P_LAB, n), in1=codeA[:],
                    op0=alu.mult, op1=alu.add)
                sc = wpool.tile([128, W], F32, tag="sc")
                nc.gpsimd.tensor_tensor(sc[:], pos[:], dn[:], alu.mult)
                nc.gpsimd.tensor_tensor(scoreA[:], scoreA[:], sc[:], alu.add)

            nc.sync.dma_start(
                oml_d.rearrange("g (ch w) -> (g ch) w", ch=CH), codeA[:])
            labU8 = spool.tile([128, W], U8)
            nc.vector.tensor_copy(labU8[:], codeA[:])
            nc.scalar.activation(scoreA[:], scoreA[:], afunc.Relu,
                                 bias=1.0, scale=-2.0)

            # md[p, c] = (labA == c+1) * scoreA, built per quarter
            omd_v = omd_d.rearrange("g (ch q wq) c -> (g ch) q (wq c)",
                                    ch=CH, q=NQ, wq=WQ)
            for q in range(NQ):
                mdt = mdpool.tile([128, WQ * C], F32, tag="mdt")
                mdt3 = mdt[:].rearrange("p (wq c) -> p wq c", c=C)
                lab_q = labU8[:, q * WQ:(q + 1) * WQ]
                sco_q = scoreA[:, q * WQ:(q + 1) * WQ]
                for c in range(C):
                    if c % 5 < 3:
                        nc.vector.scalar_tensor_tensor(
                            out=mdt3[:, :, c], in0=lab_q, scalar=float(c + 1),
                            in1=sco_q, op0=alu.is_equal, op1=alu.mult)
                    else:
                        eqt = wpool.tile([128, WQ], F32, tag="eqt")
                        nc.gpsimd.tensor_scalar(eqt[:], lab_q, float(c + 1),
                                                None, op0=alu.is_equal)
                        nc.gpsimd.tensor_tensor(mdt3[:, :, c], eqt[:], sco_q,
                                                alu.mult)
                nseg = 4 if q == NQ - 1 else 2
                seg = WQ * C // nseg
                for s in range(nseg):
                    deng = nc.sync if s % 2 == 0 else nc.scalar
                    deng.dma_start(omd_v[:, q, s * seg:(s + 1) * seg],
                                   mdt[:, s * seg:(s + 1) * seg])

    nc.compile()
    return nc


def _preprocess(gt_labels, gt_boxes, gt_ids):
    """Mirror of reference preprocessing in f32 numpy (margin-validated)."""
    f = np.float32
    b = np.asarray(gt_boxes, dtype=f)
    labels0 = np.asarray(gt_labels).astype(np.int32)
    ids0 = np.asarray(gt_ids).astype(np.int32)
    x1, y1, x2, y2 = b[..., 0], b[..., 1], b[..., 2], b[..., 3]
    cx = (x1 + x2) * f(0.5)
    cy = (y1 + y2) * f(0.5)
    w = x2 - x1
    h = y2 - y1
    area = (w * h).mean(-1)
    order = np.argsort(area, axis=1, kind="stable")
    take = np.take_along_axis
    cx = take(cx, order[:, :, None], 1)
    cy = take(cy, order[:, :, None], 1)
    w = take(w, order[:, :, None], 1)
    h = take(h, order[:, :, None], 1)
    labels = take(labels0, order, 1)
    ids = take(ids0, order[:, :, None], 1)
    valid = ((w > 0) & (h > 0)).any(-1) & (labels >= 0)          # [B,N]
    act = valid[:, :, None] & (ids != -1)                         # [B,N,T]
    iw = f(1.0) / np.maximum(w, f(MIN_WH))
    ih = f(1.0) / np.maximum(h, f(MIN_WH))
    return cx, cy, iw, ih, labels, ids, act


def build_in_maps(gt_labels, gt_boxes, gt_ids, ref_points):
    cx, cy, iw, ih, labels, ids, act = _preprocess(gt_labels, gt_boxes, gt_ids)
    rp = np.asarray(ref_points, dtype=np.float32)
    rx = np.tile(rp[:, 0].reshape(CH, W), (G, 1))
    ry = np.tile(rp[:, 1].reshape(CH, W), (G, 1))
    rxy = np.ascontiguousarray(np.concatenate([rx, ry], axis=1))

    in_maps = []
    for core in range(NCORES):
        par = np.zeros((8, 128, N), np.float32)
        for g in range(G):
            fi = core * G + g
            bb, tt = fi // T, fi % T
            rows = slice(g * CH, (g + 1) * CH)
            par[P_SXS, rows, :] = iw[bb, :, tt]
            par[P_SXB, rows, :] = -cx[bb, :, tt] * iw[bb, :, tt]
            par[P_SYS, rows, :] = ih[bb, :, tt]
            par[P_SYB, rows, :] = -cy[bb, :, tt] * ih[bb, :, tt]
            a = act[bb, :, tt].astype(np.float32)
            par[P_ACT, rows, :] = 2.0 * a - 1.0
            par[P_LAB, rows, :] = (
                labels[bb].astype(np.float32) + 1.0
                + (ids[bb, :, tt].astype(np.float32) + 1.0) * np.float32(1 / 256)
            )
        in_maps.append(dict(rxy=rxy,
                            par=np.ascontiguousarray(
                                par.transpose(1, 0, 2).reshape(128, 8 * N))))
    return in_maps


def kernel(gt_labels, gt_boxes, gt_ids, ref_points, spatial_shapes):
    if "nc" not in _CACHE:
        _CACHE["nc"] = _build_program()
    nc = _CACHE["nc"]
    in_maps = build_in_maps(gt_labels, gt_boxes, gt_ids, ref_points)
    res = run_bass_kernel_spmd(nc, in_maps, core_ids=list(range(NCORES)),
                               **_CACHE.get("run_kwargs", {}))
    _CACHE["last_result"] = res

    ml = np.empty((B, T, P), np.int32)
    mi = np.empty((B, T, P), np.int32)
    md = np.empty((B, T, P, C), np.float32)
    for core in range(NCORES):
        out = res.results[core]
        for g in range(G):
            fi = core * G + g
            bb, tt = fi // T, fi % T
            code = out["oml"][g].astype(np.float64)
            lab = np.floor(code)
            ml[bb, tt] = lab.astype(np.int32) - 1
            mi[bb, tt] = np.rint((code - lab) * 256).astype(np.int32) - 1
            md[bb, tt] = out["omd"][g]
    return ml, md, mi
